# revision 59
# baseline (speedup 1.0000x reference)
# Trainium2 Bass kernel for nn_AttnNCRFDecoder: multi-head attention + MLP
# head + 1-best Viterbi decode, data-parallel over batch across 8 NeuronCores.
#
# Device computes, per core (8 sequences): the forward model (attention,
# proj+residual+LayerNorm, MLP -> 33-tag logits, all fp32/fp32r matmuls) and
# the Viterbi value-DP over all 512 steps (delta history, on the Vector
# engine). The host does input transposition/sharding, weight folding, and
# the O(B*S*TAG) backtrace from the device-produced delta history (the
# backtrace argmaxes recompute exactly the device's fp32 sums, so decisions
# are bit-consistent with the device DP).
#
# Pipelining: the forward is chunked into two 256-position halves. Emission
# order is [per-seq: K/V + chunk-A] x8, [DP steps 1..255], [per-seq chunk-B]
# x8, [DP steps 256..511]. The DP is DVE-only and chunk-B is DVE-free
# (reciprocals on the Activation engine, elementwise muls on Pool), so the
# Vector engine runs DP part A concurrently with chunk-B's forward on
# PE/Act/Pool. This hides roughly half the forward under the DP.
#
# Key layout for the DP: 128 partitions = 4 quadrants x 32 tags; quadrant q
# holds batch pair (2q, 2q+1) with b_lo selecting the pair member. Per step:
# segmented max-reduce, seed/history adds, then a 5-round XOR-butterfly
# (stream shuffles) interleaved with the cand-block adds so no op depends on
# its immediate predecessor (hides the dependent-issue gap on the serial DVE).
import sys

for _p in ("/opt/trn_rl_repo",):
    if _p not in sys.path:
        sys.path.insert(0, _p)

import numpy as np

B, S, D = 64, 512, 768
H, DK, DV = 3, 64, 64
TAG, NBEST = 33, 8
START, STOP = TAG - 2, TAG - 1
NT = 32          # DP tag count (STOP dropped; START=31 present but dead)
NCORES = 8
BPC = B // NCORES  # sequences per core
DT = D // 128      # d-tiles
CH = 256           # forward chunk size (positions per chunk; 2 chunks)

_BUILD_CACHE = {}


def _host_prep(inputs, w_qs, w_ks, w_vs, proj_w, proj_b, ln_g, ln_b,
               lin1_w, lin1_b, lin2_w, lin2_b, transitions):
    """Build the per-core input maps (all fp32, SBUF-layout-matched)."""
    f = np.float32
    x = np.ascontiguousarray(inputs, f)                       # (B,S,D)
    # xt[b, p, dt, s] = x[b, s, dt*128+p]
    xt = np.ascontiguousarray(
        x.reshape(B, S, DT, 128).transpose(0, 3, 2, 1), f)    # (B,128,6,512)

    # K/V packing: 3 groups of 128 psum lanes, contraction over dt tiles:
    #   g0 = [k0|k1], g1 = [k2|v2], g2 = [v0|v1]
    wkv = np.zeros((128, 3 * DT * 128), f)
    for dt in range(DT):
        rows = slice(dt * 128, (dt + 1) * 128)
        blk = lambda g: slice((g * DT + dt) * 128, (g * DT + dt) * 128 + 128)
        wkv[:, blk(0)] = np.hstack([w_ks[0, rows, :], w_ks[1, rows, :]])
        wkv[:, blk(1)] = np.hstack([w_ks[2, rows, :], w_vs[2, rows, :]])
        wkv[:, blk(2)] = np.hstack([w_vs[0, rows, :], w_vs[1, rows, :]])
    # Q packing (computed per chunk): [q0|q1] 128-lane + q2 64-lane
    wq01 = np.zeros((128, DT * 128), f)
    wq2 = np.zeros((128, DT * 128), f)
    for dt in range(DT):
        rows = slice(dt * 128, (dt + 1) * 128)
        wq01[:, dt * 128:dt * 128 + 64] = w_qs[0, rows, :]
        wq01[:, dt * 128 + 64:(dt + 1) * 128] = w_qs[1, rows, :]
        wq2[:, dt * 128:dt * 128 + 64] = w_qs[2, rows, :]
        wq2[:, dt * 128 + 64:(dt + 1) * 128] = w_qs[2, rows, :]

    projbt = np.ascontiguousarray(proj_b.reshape(DT, 128).T, f)  # (128,DT)
    wp_a = np.ascontiguousarray(proj_w[0:128, :], f)          # (128,768)
    wp_b = np.ascontiguousarray(proj_w[128:192, :], f)        # (64,768)
    projb = np.ascontiguousarray(proj_b.reshape(1, D), f)

    w1eff = (ln_g[:, None] * lin1_w).astype(f)                # (768,384)
    w1s = np.zeros((128, DT * 384), f)
    for dt in range(DT):
        w1s[:, dt * 384:(dt + 1) * 384] = w1eff[dt * 128:(dt + 1) * 128, :]
    csw1 = np.ascontiguousarray(w1eff.sum(0).reshape(1, 384), f)
    b1eff = (ln_b @ lin1_w + lin1_b).astype(f)                # (384,)
    b1 = np.ascontiguousarray(b1eff.reshape(3, 128).T, f)     # (128,3)

    w2s = np.zeros((128, 3 * 128), f)
    for kt in range(3):
        blk = lin2_w[kt * 128:(kt + 1) * 128, 0:NT]
        w2s[:, kt * 128:(kt + 1) * 128] = np.tile(blk, (1, 4))
    l2b = np.tile(lin2_b[0:NT], 4).reshape(128, 1).astype(f)

    tr = transitions.astype(f)
    # XOR-butterfly layout: transx[32q+to, 2c+b_lo] = trans[to^c, to]
    transx = np.zeros((128, 2 * NT), f)
    for q in range(4):
        for to in range(NT):
            for c in range(NT):
                transx[q * 32 + to, 2 * c] = tr[to ^ c, to]
                transx[q * 32 + to, 2 * c + 1] = tr[to ^ c, to]
    transstart = np.tile(tr[START, 0:NT], 4).reshape(128, 1).astype(f)

    hlc = np.full((1, 1), 0.5 * np.log(1.0 / (D - 1)), f)
    epsc = np.full((1, 1), 1e-3, f)
    ident = np.eye(128, dtype=f)
    ident64 = np.vstack([np.eye(64, dtype=f), np.eye(64, dtype=f)])  # (128,64)
    onesrow = np.ones((1, S), f)
    onescol = np.ones((128, 1), f)

    shared = dict(hlc=hlc, epsc=epsc, wkv=wkv, wq01=wq01, wq2=wq2, wp_a=wp_a, wp_b=wp_b,
                  projb=projb, projbt=projbt, w1s=w1s, csw1=csw1, b1=b1, w2s=w2s, l2b=l2b,
                  transx=transx, transstart=transstart, ident=ident,
                  ident64=ident64, onesrow=onesrow, onescol=onescol)
    in_maps = []
    for c in range(NCORES):
        m = dict(shared)
        m["xt"] = np.ascontiguousarray(xt[c * BPC:(c + 1) * BPC])
        in_maps.append(m)
    return in_maps


def build_nc():
    import concourse.bass as bass
    import concourse.mybir as mybir
    import concourse.tile as tile
    from concourse import bacc

    f32 = mybir.dt.float32
    f32r = mybir.dt.float32r
    ADD = mybir.AluOpType.add
    MULT = mybir.AluOpType.mult
    MAX = mybir.AluOpType.max
    AF = mybir.ActivationFunctionType

    nc = bacc.Bacc(None, target_bir_lowering=False, debug=False)

    def mm(out, lhsT, rhs, start, stop):
        nc.tensor.matmul(out, lhsT, rhs, start=start, stop=stop)


    with tile.TileContext(nc) as tc:
        from contextlib import ExitStack
        ctx = ExitStack()
        with ctx:
            dram = ctx.enter_context(tc.tile_pool(name="dram", bufs=1, space="DRAM"))

            def din(name, shape, dt_=None):
                return dram.tile(shape, dt_ or f32, kind="ExternalInput",
                                 name=name, uniquify=False)

            xt_d = din("xt", (BPC, 128, DT, S), f32r)
            wkv_d = din("wkv", (128, 3 * DT * 128), f32r)
            wq01_d = din("wq01", (128, DT * 128), f32r)
            wq2_d = din("wq2", (128, DT * 128), f32r)
            wpa_d = din("wp_a", (128, D), f32r)
            wpb_d = din("wp_b", (64, D), f32r)
            projb_d = din("projb", (1, D), f32r)
            projbt_d = din("projbt", (128, DT))
            w1s_d = din("w1s", (128, DT * 384), f32r)
            csw1_d = din("csw1", (1, 384), f32r)
            b1_d = din("b1", (128, 3))
            w2s_d = din("w2s", (128, 3 * 128), f32r)
            l2b_d = din("l2b", (128, 1))
            transx_d = din("transx", (128, 2 * NT))
            transstart_d = din("transstart", (128, 1))
            ident_d = din("ident", (128, 128), f32r)
            ident64_d = din("ident64", (128, 64), f32r)
            onesrow_d = din("onesrow", (1, S), f32r)
            onescol_d = din("onescol", (128, 1), f32r)
            hlc_d = din("hlc", (1, 1))
            epsc_d = din("epsc", (1, 1))
            dh_d = dram.tile((128, 2 * S), f32, kind="ExternalOutput",
                             name="dh", uniquify=False)

            # fire the first two xt loads before the const block so the
            # first K/V matmuls aren't gated on the whole weight transfer
            xtp_early = None
            xtp = ctx.enter_context(tc.tile_pool(name="xtp", bufs=3))

            def load_xt(s):
                xt_s = xtp.tile((128, DT * S), f32r, tag="xt", name="xt_s")
                nc.sync.dma_start(out=xt_s[:], in_=xt_d[s])
                return xt_s

            import os as _os
            _skip_fwd = _os.environ.get("KSKIP_FWD") == "1"
            _skip_dp = _os.environ.get("KSKIP_DP") == "1"

            cp = ctx.enter_context(tc.tile_pool(name="consts", bufs=1))
            wkv_s = cp.tile((128, 3 * DT * 128), f32r)
            wq01_s = cp.tile((128, DT * 128), f32r)
            wq2_s = cp.tile((128, DT * 128), f32r)
            wpa_s = cp.tile((128, D), f32r)
            wpb_s = cp.tile((64, D), f32r)
            projb_s = cp.tile((1, D), f32r)
            projbt_s = cp.tile((128, DT), f32)
            w1s_s = cp.tile((128, DT * 384), f32r)
            csw1_s = cp.tile((1, 384), f32r)
            b1_s = cp.tile((128, 3), f32)
            w2s_s = cp.tile((128, 3 * 128), f32r)
            l2b_s = cp.tile((128, 1), f32)
            transx_s = cp.tile((128, 2 * NT), f32)
            transstart_s = cp.tile((128, 1), f32)
            ident_s = cp.tile((128, 128), f32r)
            ident64_s = cp.tile((128, 64), f32r)
            onesrow_s = cp.tile((1, S), f32r)
            onescol_s = cp.tile((128, 1), f32r)
            hlc_s = cp.tile((1, 1), f32)
            epsc_s = cp.tile((1, 1), f32)
            # wkv first, then the two xt prefetches: the first K/V
            # matmuls need exactly these three transfers
            nc.sync.dma_start(out=wkv_s[:], in_=wkv_d[:])
            xts = [None] * BPC
            if not _skip_fwd:
                xts[0] = load_xt(0)
                xts[1] = load_xt(1)
            for sb, dr in [(wq01_s, wq01_d), (wq2_s, wq2_d),
                           (wpa_s, wpa_d), (wpb_s, wpb_d), (projb_s, projb_d),
                           (projbt_s, projbt_d),
                           (w1s_s, w1s_d), (csw1_s, csw1_d), (b1_s, b1_d),
                           (w2s_s, w2s_d), (l2b_s, l2b_d),
                           (transx_s, transx_d), (transstart_s, transstart_d),
                           (ident_s, ident_d), (ident64_s, ident64_d),
                           (onesrow_s, onesrow_d),
                           (onescol_s, onescol_d), (hlc_s, hlc_d),
                           (epsc_s, epsc_d)]:
                nc.sync.dma_start(out=sb[:], in_=dr[:])

            # persistent per-seq K / V (resident across both chunks)
            kvp = ctx.enter_context(tc.tile_pool(name="kv", bufs=1))
            kT01 = [kvp.tile((128, S), f32r, name=f"kT01_{s}", tag=f"kT01_{s}")
                    for s in range(BPC)]
            kT2p = [kvp.tile((128, S), f32r, name=f"kT2_{i}", tag=f"kT2_{i}")
                    for i in range(BPC // 2)]
            v_sb = [[kvp.tile((128, 4 * 64), f32r, name=f"v_{s}_{h}", tag=f"v_{s}_{h}")
                     for h in range(H)] for s in range(BPC)]

            # persistent DP state
            dpp = ctx.enter_context(tc.tile_pool(name="dp", bufs=1))
            logit_rep = dpp.tile((128, 2 * S), f32)   # [(q,to), (t,b_lo)]
            combo = dpp.tile((128, 2 * S + 2 * NT), f32)
            dh_sb = combo[:, 0:2 * S]
            Wt0 = 2 * S  # W base column inside combo
            candt = dpp.tile((128, 2 * NT), f32)
            ndr = dpp.tile((128, 2), f32)
            ndrB = dpp.tile((128, 2), f32)

            # rotating pools
            xbp = ctx.enter_context(tc.tile_pool(name="xbp", bufs=2))
            sbA = ctx.enter_context(tc.tile_pool(name="sbA", bufs=2))
            sbw = ctx.enter_context(tc.tile_pool(name="sbw", bufs=2))
            sbs = ctx.enter_context(tc.tile_pool(name="sbs", bufs=5))
            sbr = ctx.enter_context(tc.tile_pool(name="sbr", bufs=3))
            sbn = ctx.enter_context(tc.tile_pool(name="sbn", bufs=2))
            psC = ctx.enter_context(tc.tile_pool(name="psC", bufs=3, space="PSUM"))
            psAV = ctx.enter_context(tc.tile_pool(name="psAV", bufs=2, space="PSUM"))
            psS = ctx.enter_context(tc.tile_pool(name="psS", bufs=2, space="PSUM"))
            psAcc = ctx.enter_context(tc.tile_pool(name="psAcc", bufs=1, space="PSUM"))

            def kv_phase(s, xt_s):
                """Compute K (all S) + V (transposed) for seq s."""
                r0 = 64 * (s % 2)
                vt01 = sbA.tile((128, S), f32r, tag="vt01")
                vt2 = sbA.tile((64, S), f32r, tag="vt2")
                for g in range(3):
                    for hf in range(2):
                        cols = slice(hf * CH, (hf + 1) * CH)
                        p = psC.tile((128, CH), f32, tag="mm")
                        for dt in range(DT):
                            mm(p[:],
                               wkv_s[:, (g * DT + dt) * 128:(g * DT + dt + 1) * 128],
                               xt_s[:, dt * S + hf * CH:dt * S + (hf + 1) * CH],
                               dt == 0, dt == DT - 1)
                        if g == 0:
                            nc.vector.tensor_copy(kT01[s][:, cols], p[:])
                        elif g == 1:
                            nc.vector.tensor_copy(
                                kT2p[s // 2][r0:r0 + 64, cols], p[0:64, :])
                            nc.vector.tensor_copy(vt2[:, cols], p[64:128, :])
                        else:
                            nc.vector.tensor_copy(vt01[:, cols], p[:])
                for h in range(H):
                    vsrc = vt01 if h < 2 else vt2
                    rr = 64 * h if h < 2 else 0
                    for st in range(4):
                        pvt = psC.tile((128, CH), f32r, tag="mm")
                        nc.tensor.transpose(
                            pvt[:, 0:64], vsrc[rr:rr + 64, st * 128:(st + 1) * 128],
                            ident64_s[rr:rr + 64, 0:64])
                        nc.vector.tensor_copy(
                            v_sb[s][h][:, st * 64:(st + 1) * 64], pvt[:, 0:64])

            def q_part(s, xt_col, use_dve):
                """Q matmuls + PSUM->SBUF copies, emitted early so the qT
                copies sit ahead of the serial LN chain in the DVE queue."""
                pq = psC.tile((128, CH), f32, tag="mm")
                for dt in range(DT):
                    mm(pq[:], wq01_s[:, dt * 128:(dt + 1) * 128], xt_col(dt),
                       dt == 0, dt == DT - 1)
                cpy = nc.vector.tensor_copy if use_dve else nc.scalar.copy
                qT01 = sbw.tile((128, CH), f32r, tag="qT01")
                cpy(qT01[:], pq[:])
                pq2 = psC.tile((128, CH), f32, tag="mm")
                for dt in range(DT):
                    mm(pq2[:], wq2_s[:, dt * 128:(dt + 1) * 128], xt_col(dt),
                       dt == 0, dt == DT - 1)
                qT2 = sbw.tile((128, CH), f32r, tag="qT2")
                cpy(qT2[:], pq2[:])
                return qT01, qT2

            def chunk_attn(s, qT, use_dve):
                """Attention for one chunk of seq s; returns oT parts.
                use_dve=False keeps the chunk off the Vector engine."""
                qT01, qT2 = qT
                cpy = nc.vector.tensor_copy if use_dve else nc.scalar.copy
                r2 = 64 * (s % 2)
                qTs = [qT01[0:64, :], qT01[64:128, :],
                       qT2[r2:r2 + 64, :]]
                kTs = [kT01[s][0:64, :], kT01[s][64:128, :],
                       kT2p[s // 2][r2:r2 + 64, :]]

                oT_a = sbw.tile((128, CH), f32r, tag="oT_a")
                oT_b = sbw.tile((64, CH), f32r, tag="oT_b")
                if use_dve:
                    # head-staggered: all 12 score matmuls first, then the
                    # per-head denominator+reciprocal, then all AV matmuls --
                    # each PE consumer trails its ACT/DVE producer by a full
                    # head of independent matmuls
                    pTs, rcps, pavs_ = [], [], []
                    for h in range(H):
                        pT = sbw.tile((128, 4 * CH), f32r, tag="pT", bufs=3)
                        for jt in range(4):
                            pst = psC.tile((128, CH), f32, tag="mm")
                            mm(pst[:], kTs[h][:, jt * 128:(jt + 1) * 128],
                               qTs[h], True, True)
                            nc.scalar.activation(pT[:, jt * CH:(jt + 1) * CH],
                                                 pst[:], AF.Exp, scale=0.125)
                        pTs.append(pT)
                    for h in range(H):
                        psum = psS.tile((1, CH), f32, tag="psum", bufs=1)
                        for jt in range(4):
                            mm(psum[:], onescol_s[:],
                               pTs[h][:, jt * CH:(jt + 1) * CH],
                               jt == 0, jt == 3)
                        rcp = sbr.tile((1, CH), f32r, tag="rcp")
                        with nc.allow_low_precision("fp32r feed for matmul"):
                            nc.vector.reciprocal(rcp[:], psum[:])
                        rcps.append(rcp)
                    for h in range(H):
                        pav = psAV.tile((64, CH), f32, tag="av", bufs=3)
                        for st in range(4):
                            mm(pav[:], v_sb[s][h][:, st * 64:(st + 1) * 64],
                               pTs[h][:, st * CH:(st + 1) * CH],
                               st == 0, st == 3)
                        pavs_.append(pav)
                    for h in range(H):
                        prr = psC.tile((128, CH), f32, tag="mm")
                        mm(prr[0:64, :], onesrow_s[:, 0:64], rcps[h][:],
                           True, True)
                        rrep = sbw.tile((64, CH), f32, tag="rrep", bufs=1)
                        cpy(rrep[:], prr[0:64, :])
                        odst = (oT_a[64 * h:64 * (h + 1), :] if h < 2
                                else oT_b[:])
                        nc.vector.tensor_mul(odst, pavs_[h][:], rrep[:])
                    return oT_a, oT_b
                for h in range(H):
                    pT = sbw.tile((128, 4 * CH), f32r, tag="pT", bufs=3)
                    psum = psS.tile((1, CH), f32, tag="psum", bufs=1)
                    for jt in range(4):
                        pst = psC.tile((128, CH), f32, tag="mm")
                        mm(pst[:], kTs[h][:, jt * 128:(jt + 1) * 128],
                           qTs[h], True, True)
                        nc.scalar.activation(pT[:, jt * CH:(jt + 1) * CH],
                                             pst[:], AF.Exp, scale=0.125)
                    for jt in range(4):
                        mm(psum[:], onescol_s[:], pT[:, jt * CH:(jt + 1) * CH],
                           jt == 0, jt == 3)
                    pav = psAV.tile((64, CH), f32, tag="av", bufs=3)
                    for st in range(4):
                        mm(pav[:], v_sb[s][h][:, st * 64:(st + 1) * 64],
                           pT[:, st * CH:(st + 1) * CH], st == 0, st == 3)
                    # 1/x as exp(-ln(x)): keeps the chunk off the DVE
                    # (Ln and Exp share one activation table)
                    rcp = sbr.tile((1, CH), f32r, tag="rcp")
                    lnp = sbr.tile((1, CH), f32, tag="rcp")
                    nc.scalar.activation(lnp[:], psum[:], AF.Ln)
                    nc.scalar.activation(rcp[:], lnp[:], AF.Exp, scale=-1.0)
                    prr = psC.tile((128, CH), f32, tag="mm")
                    mm(prr[0:64, :], onesrow_s[:, 0:64], rcp[:], True, True)
                    rrep = sbw.tile((64, CH), f32, tag="rrep", bufs=1)
                    cpy(rrep[:], prr[0:64, :])
                    odst = oT_a[64 * h:64 * (h + 1), :] if h < 2 else oT_b[:]
                    # Pool has no PSUM port: stage pav through SBUF
                    pavs = sbw.tile((64, CH), f32, tag="pavs", bufs=1)
                    nc.scalar.copy(pavs[:], pav[:])
                    nc.gpsimd.tensor_mul(odst, pavs[:], rrep[:])
                return oT_a, oT_b

            def mlp_stats(s, c0, xt_col, use_dve, oT):
                """proj + residual + LN stats chain through rln for one
                chunk of seq s. Returns state for mlp_tail. The serial LN
                chain (ACT/Pool/DVE hops + table loads) drains while the
                next seq's attention matmuls keep the PE busy."""
                oT_a, oT_b = oT
                cpz = nc.scalar.copy  # z gates the stats matmuls: keep low-latency

                # ---- proj + residual + LN stats ----
                z_sb = sbw.tile((128, DT * CH), f32r, tag="z", bufs=1)
                acc = psAcc.tile((1, 2 * CH), f32, tag="acc")
                pmu = acc[:, 0:CH]
                ps2_ = acc[:, CH:2 * CH]
                z2s = []
                for mt in range(DT):
                    pz = psC.tile((128, CH), f32, tag="mm")
                    if use_dve:
                        # residual + bias fused into the PSUM->SBUF move:
                        # z = (proj + b) + x on the DVE
                        mm(pz[:], wpa_s[:, mt * 128:(mt + 1) * 128], oT_a[:],
                           True, False)
                        mm(pz[:], wpb_s[:, mt * 128:(mt + 1) * 128], oT_b[:],
                           False, True)
                        nc.vector.scalar_tensor_tensor(
                            z_sb[:, mt * CH:(mt + 1) * CH], pz[:],
                            projbt_s[:, mt:mt + 1], xt_col(mt), ADD, ADD)
                    else:
                        mm(pz[:], wpa_s[:, mt * 128:(mt + 1) * 128], oT_a[:],
                           True, False)
                        mm(pz[:], wpb_s[:, mt * 128:(mt + 1) * 128], oT_b[:],
                           False, False)
                        mm(pz[:], ident_s[:], xt_col(mt), False, False)
                        mm(pz[:], projb_s[:, mt * 128:(mt + 1) * 128],
                           onesrow_s[:, 0:CH], False, True)
                        cpz(z_sb[:, mt * CH:(mt + 1) * CH], pz[:])
                    z2 = sbw.tile((128, CH), f32r, tag="z2", bufs=6)
                    nc.gpsimd.tensor_mul(z2[:], z_sb[:, mt * CH:(mt + 1) * CH],
                                         z_sb[:, mt * CH:(mt + 1) * CH])
                    z2s.append(z2)
                return z_sb, z2s, pmu, ps2_, use_dve, xt_col

            def mlp_stats2(state):
                """Stat matmuls + LN chain; emitted after the next seq's
                q matmuls so the PE has cover while the z stt/z2 chain on
                DVE/Pool drains."""
                z_sb, z2s, pmu, ps2_, use_dve, xt_col = state
                for mt in range(DT):
                    mm(pmu, onescol_s[:], z_sb[:, mt * CH:(mt + 1) * CH],
                       mt == 0, mt == DT - 1)
                for mt in range(DT):
                    mm(ps2_, onescol_s[:], z2s[mt][:], mt == 0, mt == DT - 1)

                negmu = sbn.tile((1, CH), f32r, tag="negmu")
                nc.scalar.mul(negmu[:], pmu, -1.0 / D)
                varnum = sbs.tile((1, CH), f32, tag="st1")
                if use_dve:
                    mu2 = sbs.tile((1, CH), f32, tag="st1")
                    nc.gpsimd.tensor_mul(mu2[:], negmu[:], negmu[:])
                    nc.vector.scalar_tensor_tensor(varnum[:], mu2[:], float(-D),
                                                   ps2_, MULT, ADD)
                else:
                    # Pool cannot touch PSUM or run tensor_scalar: compute
                    # Dmu^2 = (sqrt(D)*mu)^2 on Pool from an ACT-scaled mu,
                    # then varnum = ps2 - Dmu^2 as a plain tensor_sub.
                    negmud = sbs.tile((1, CH), f32, tag="st1")
                    nc.scalar.mul(negmud[:], pmu, -1.0 / np.sqrt(D))
                    mu2d = sbs.tile((1, CH), f32, tag="st1")
                    nc.gpsimd.tensor_mul(mu2d[:], negmud[:], negmud[:])
                    ps2s = sbs.tile((1, CH), f32, tag="st1")
                    nc.scalar.copy(ps2s[:], ps2_)
                    nc.gpsimd.tensor_sub(varnum[:], ps2s[:], mu2d[:])
                # sig = sqrt(varnum/(D-1)) = exp(0.5 ln varnum + 0.5 ln 1/(D-1))
                # (keeps Sqrt's separate activation table out of the rotation)
                lnv = sbs.tile((1, CH), f32, tag="st1")
                nc.scalar.activation(lnv[:], varnum[:], AF.Ln)
                sig = sbs.tile((1, CH), f32, tag="st1")
                nc.scalar.activation(sig[:], lnv[:], AF.Exp, scale=0.5,
                                     bias=hlc_s[:])
                rln = sbs.tile((1, CH), f32r, tag="st1")
                if use_dve:
                    sige = sbs.tile((1, CH), f32, tag="st1")
                    nc.vector.tensor_scalar_add(sige[:], sig[:], 1e-3)
                    with nc.allow_low_precision("fp32r feed for matmul"):
                        nc.vector.reciprocal(rln[:], sige[:])
                else:
                    # rln = 1/(sig+eps) = exp(-ln(sig+eps)); eps rides the
                    # Ln activation's per-partition bias
                    lns = sbs.tile((1, CH), f32, tag="st1")
                    nc.scalar.activation(lns[:], sig[:], AF.Ln, bias=epsc_s[:])
                    nc.scalar.activation(rln[:], lns[:], AF.Exp, scale=-1.0)
                return z_sb, negmu, rln

            def mlp_tail(s, c0, use_dve, state):
                """rln broadcast + lin1 + lin2 + logit write."""
                q32 = 32 * (s // 2)
                b_lo = s % 2
                z_sb, negmu, rln = state
                cpt = nc.scalar.copy  # ACT queue is shorter than DVE's here
                prl = psC.tile((128, CH), f32, tag="mm")
                mm(prl[:], onesrow_s[:, 0:128], rln[:], True, True)
                rln_rep = sbw.tile((128, CH), f32, tag="rlnrep")
                cpt(rln_rep[:], prl[:])

                # ---- lin1 (LN folded) + tanh ----
                hT = sbw.tile((128, 3 * CH), f32r, tag="hT")
                for mt in range(3):
                    pg = psC.tile((128, CH), f32, tag="mm")
                    for kt in range(DT):
                        mm(pg[:], w1s_s[:, kt * 384 + mt * 128:kt * 384 + (mt + 1) * 128],
                           z_sb[:, kt * CH:(kt + 1) * CH], kt == 0, False)
                    mm(pg[:], csw1_s[:, mt * 128:(mt + 1) * 128], negmu[:],
                       False, True)
                    g_sb = sbw.tile((128, CH), f32, tag="g")
                    cpt(g_sb[:], pg[:])
                    gr = sbw.tile((128, CH), f32, tag="gr")
                    nc.gpsimd.tensor_mul(gr[:], g_sb[:], rln_rep[:])
                    nc.scalar.activation(hT[:, mt * CH:(mt + 1) * CH], gr[:],
                                         AF.Tanh, bias=b1_s[:, mt:mt + 1])

                # ---- lin2 -> logits into DP layout ----
                plg = psC.tile((128, CH), f32, tag="mm")
                for kt in range(3):
                    mm(plg[:], w2s_s[:, kt * 128:(kt + 1) * 128],
                       hT[:, kt * CH:(kt + 1) * CH], kt == 0, kt == 2)
                nc.scalar.activation(
                    logit_rep[q32:q32 + NT, 2 * c0 + b_lo:2 * (c0 + CH):2],
                    plg[q32:q32 + NT, :],
                    AF.Identity, bias=l2b_s[q32:q32 + NT, :])

            # ---- Viterbi value DP (interleaved 14-op schedule) ----
            def butterfly():
                for k in range(5):
                    w = 2 << k
                    nc.vector.stream_shuffle(
                        combo[:, Wt0 + w:Wt0 + 2 * w], combo[:, Wt0:Wt0 + w],
                        [i ^ (1 << k) for i in range(32)])

            red_in = bass.AP(
                tensor=candt[:].tensor, offset=candt[:].offset,
                ap=[[candt[:].ap[0][0], 128], [1, 2], [2, NT]])

            def dp_seed():
                nc.vector.tensor_scalar_add(combo[:, Wt0:Wt0 + 2],
                                            logit_rep[:, 0:2],
                                            transstart_s[:])
                nc.vector.tensor_copy(combo[:, 0:2], combo[:, Wt0:Wt0 + 2])
                butterfly()
                nc.vector.tensor_add(candt[:], combo[:, Wt0:Wt0 + 2 * NT],
                                     transx_s[:])

            def dp_part(t0, t1):
                for t in range(t0, t1):
                    nc.vector.tensor_reduce(ndr[:], red_in,
                                            mybir.AxisListType.X, MAX)
                    nc.vector.tensor_add(combo[:, Wt0:Wt0 + 2], ndr[:],
                                         logit_rep[:, 2 * t:2 * t + 2])
                    nc.vector.tensor_add(combo[:, 2 * t:2 * t + 2], ndr[:],
                                         logit_rep[:, 2 * t:2 * t + 2])
                    if t < S - 1:
                        sh = [(2 << k, 1 << k) for k in range(5)]
                        w, m = sh[0]
                        nc.vector.stream_shuffle(
                            combo[:, Wt0 + w:Wt0 + 2 * w],
                            combo[:, Wt0:Wt0 + w],
                            [i ^ m for i in range(32)])
                        nc.vector.tensor_add(candt[:, 0:2],
                                             combo[:, Wt0:Wt0 + 2],
                                             transx_s[:, 0:2])
                        for k in range(1, 5):
                            w, m = sh[k]
                            nc.vector.stream_shuffle(
                                combo[:, Wt0 + w:Wt0 + 2 * w],
                                combo[:, Wt0:Wt0 + w],
                                [i ^ m for i in range(32)])
                            pw, _ = sh[k - 1]
                            nc.vector.tensor_add(
                                candt[:, pw:2 * pw],
                                combo[:, Wt0 + pw:Wt0 + 2 * pw],
                                transx_s[:, pw:2 * pw])
                        w = 32
                        nc.vector.tensor_add(candt[:, w:2 * w],
                                             combo[:, Wt0 + w:Wt0 + 2 * w],
                                             transx_s[:, w:2 * w])

            # ================= program =================
            if _skip_fwd:
                nc.vector.memset(logit_rep[:], 0.0)
            else:
                # Phase A: per seq K/V + chunk A (positions [0, CH)).
                # Software-pipelined: K/V two seqs ahead, attention one seq
                # ahead of the proj/MLP tail, so the in-order PE always has
                # independent matmuls while a seq's cross-engine chain drains.
                oTs = [None] * BPC

                def xcolA(s):
                    return lambda dt, x=xts[s]: x[:, dt * S:dt * S + CH]

                xbs = [None] * BPC
                oTb = [None] * BPC

                def xcolB(s):
                    return lambda dt, x=xbs[s]: x[:, dt * CH:(dt + 1) * CH]

                def loadB(s):
                    xb = xbp.tile((128, DT * CH), f32r, tag="xtb",
                                  name="xtb")
                    nc.sync.dma_start(out=xb[:], in_=xt_d[s][:, :, CH:S])
                    return xb

                kv_phase(0, xts[0])
                qA = q_part(0, xcolA(0), use_dve=True)
                oTs[0] = chunk_attn(0, qA, use_dve=True)
                for s in range(BPC):
                    if s + 2 < BPC:
                        xts[s + 2] = load_xt(s + 2)
                    if s + 1 < BPC:
                        kv_phase(s + 1, xts[s + 1])
                    mid = mlp_stats(s, 0, xcolA(s), True, oTs[s])
                    if s + 1 < BPC:
                        qA = q_part(s + 1, xcolA(s + 1), use_dve=True)
                    st = mlp_stats2(mid)
                    if s + 1 < BPC:
                        oTs[s + 1] = chunk_attn(s + 1, qA, use_dve=True)
                    mlp_tail(s, 0, True, st)

            if _skip_dp:
                nc.vector.tensor_copy(dh_sb[:], logit_rep[:])
                dp_seed()
            else:
                dp_seed()
                dp_part(1, CH)
                # dh cols [0, 2*CH) are final after DP part A: ship them
                # while DP part B runs so no DMA tail remains at the end
                nc.sync.dma_start(out=dh_d[:, 0:2 * CH],
                                  in_=combo[:, 0:2 * CH])

            if not _skip_fwd:
                # Phase B: chunk B (positions [CH, S)), DVE-free; overlaps
                # DP part A on the Vector engine.
                xbs[0] = loadB(0)
                qB0 = q_part(0, xcolB(0), use_dve=False)
                oTb[0] = chunk_attn(0, qB0, use_dve=False)
                for s in range(BPC):
                    mid = mlp_stats(s, CH, xcolB(s), False, oTb[s])
                    if s + 1 < BPC:
                        xbs[s + 1] = loadB(s + 1)
                        qB = q_part(s + 1, xcolB(s + 1), use_dve=False)
                    st = mlp_stats2(mid)
                    if s + 1 < BPC:
                        oTb[s + 1] = chunk_attn(s + 1, qB, use_dve=False)
                    mlp_tail(s, CH, False, st)

            if not _skip_dp:
                dp_part(CH, S)
                nc.sync.dma_start(out=dh_d[:, 2 * CH:2 * S],
                                  in_=combo[:, 2 * CH:2 * S])
            else:
                nc.sync.dma_start(out=dh_d[:], in_=combo[:, 0:2 * S])

    nc.compile()
    return nc


def _get_nc():
    if "nc" not in _BUILD_CACHE:
        _BUILD_CACHE["nc"] = build_nc()
    return _BUILD_CACHE["nc"]


def _backtrace(dh_all, lengths, transitions):
    """dh_all: (B, S, NT) device delta history. Vectorized over batch."""
    f = np.float32
    tr = transitions[0:NT, 0:NT].astype(f)           # [from, to]
    out = np.zeros((B, S), np.int32)
    # final tag per b at its own L-1
    last = dh_all[np.arange(B), lengths - 1, :] + transitions[0:NT, STOP][None, :]
    tag = last.argmax(1).astype(np.int64)            # (B,)
    out[:, S - 1] = tag
    out[np.arange(B), lengths - 1] = tag
    # walk all b in lockstep over t; only update b where t <= L-2
    cur = tag.copy()
    for t in range(S - 2, -1, -1):
        cand = dh_all[:, t, :] + tr[:, cur].T        # (B, NT) fp32
        prev = cand.argmax(1)
        active = t <= lengths - 2
        cur = np.where(active, prev, cur)
        out[:, t] = np.where(active, cur, out[:, t])
    return out


def kernel(**inputs):
    from concourse.bass_utils import run_bass_kernel_spmd

    args = {k: np.asarray(v) for k, v in inputs.items()}
    in_maps = _host_prep(
        args["inputs"], args["w_qs"], args["w_ks"], args["w_vs"],
        args["proj_w"], args["proj_b"], args["ln_g"], args["ln_b"],
        args["lin1_w"], args["lin1_b"], args["lin2_w"], args["lin2_b"],
        args["transitions"])

    nc = _get_nc()
    res = run_bass_kernel_spmd(nc, in_maps, core_ids=list(range(NCORES)))

    dh_all = np.zeros((B, S, NT), np.float32)
    for c in range(NCORES):
        dh = res.results[c]["dh"]                    # (128, 2S)
        # dh[32q+to, 2t+b_lo] -> delta[b=2q+b_lo, t, to]
        d = dh.reshape(4, 32, S, 2).transpose(0, 3, 2, 1)  # (q, b_lo, t, to)
        dh_all[c * BPC:(c + 1) * BPC] = d.reshape(BPC, S, NT)

    lengths = np.asarray(args["labels_mask"]).astype(np.int64).sum(1)
    return _backtrace(dh_all, lengths, args["transitions"])
